# revision 25
# baseline (speedup 1.0000x reference)
"""AdaGAE forward on 8 TRN2 NeuronCores.

Computes, for A = norm_adj_matrix [8192, 8192]:
    h    = relu(A @ (X @ W1))           [n, 24]
    emb  = A @ (h @ W2)                 [n, 12]
    out  = softmax(-pairwise_sq_dists(emb), axis=1) + 1e-10

Key algebra: softmax is shift-invariant per row and the relu on squared
distances is a numerical no-op, so
    out[i, :] = softmax_j(2*<emb_i, emb_j> - |emb_j|^2) + eps
with the row-constant |emb_i|^2 cancelled. The -|emb_j|^2 bias is folded
into the Gram matmul as two extra contraction rows (ones x -sq_hi/lo), so
the whole logits matrix comes out of the PE in one 34-row fp16 matmul per
tile, straight into exp on the ScalarE with a free per-row accumulator,
then a fused multiply-add (x*recip + eps) normalize on DVE in bf16.

Distribution: row-shard A across 8 cores. Each core stages its shard as
A_shard.T in fp16 (fits SBUF entirely -> A is read from HBM exactly once),
computes hT/embT for its rows, and two tiny AllGathers (hW2, then the
emb+sq halves, pipelined against the second A-matmul and the Gram phase)
make the full embedding visible everywhere. fp16 is used for the big
streams (bf16 is too coarse for the softmax logits); XW1/hW2/emb are
plain fp16, -|emb|^2 ships hi+lo fp16 (its error is column-systematic),
and everything accumulates in f32 PSUM. ~210 us on silicon, L2 err 2.9e-3.
"""
import sys

for _p in ("/opt/trn_rl_repo", "/opt/trn_rl_repo/concourse"):
    if _p not in sys.path:
        sys.path.insert(0, _p)

import numpy as np

from concourse import bacc, mybir, tile
from concourse import bass_utils
from concourse.masks import make_identity

F32 = mybir.dt.float32
F16 = mybir.dt.float16
BF16 = mybir.dt.bfloat16
ADD = mybir.AluOpType.add
SUB = mybir.AluOpType.subtract
MULT = mybir.AluOpType.mult
AF = mybir.ActivationFunctionType

N = 8192
NCORES = 8
SH = N // NCORES          # 1024 rows per core
P = 128
KB = N // P               # 64 contraction blocks
D_IN, D_MID, D_EMB = 32, 24, 12
KAUG = 14                 # rows 0:12 = 2e_hi | e_hi, rows 12:14 = ones | -sq_hi/lo
EPS = 1e-10
RG = [list(range(NCORES))]

_NC = None


def _emit(nc, tc, a_t, xw1_in, w2, out):
    HS = SH // 2              # 512: shard-row half for collective overlap
    dram_cm = tc.tile_pool(name="dram", bufs=1, space="DRAM")
    dram = dram_cm.__enter__()
    sbp_cm = tc.tile_pool(name="sbp", bufs=1)
    sbp = sbp_cm.__enter__()

    # persists into phase 3: the Gram matmul's stationary operand. Engine
    # writes must start at a partition multiple of 32, so the ones rows
    # (12:14) are set by memsetting all 14 rows; rows 0:12 are overwritten
    # with 2*e_hi in phase 2.
    lhsT_sb = sbp.tile([KAUG, SH], F16, name="lhsT_sb")
    nc.vector.memset(lhsT_sb[:], 1.0)
    s3a_box = {}

    with tc.tile_pool(name="s12", bufs=1) as s12:
        # ---- small inputs ----
        w2_sb = s12.tile([D_MID, D_EMB], F32, name="w2_sb")
        nc.sync.dma_start(w2_sb[:], w2[:])
        ident = s12.tile([P, P], F32, name="ident")
        make_identity(nc, ident)

        # tiny first collective: the first AllGather on the CC stream pays
        # ~2x duration (NRT first-call staging), so burn that cost on a 64B
        # op during the A-load window instead of on AG1
        warm_sb = s12.tile([1, 16], F32, name="warm_sb")
        nc.vector.memset(warm_sb[:], 0.0)
        warm_in = dram.tile([1, 16], F32, name="warm_in")
        nc.sync.dma_start(warm_in[:], warm_sb[:])
        warm_out = dram.tile([NCORES, 1, 16], F32, name="warm_out")
        nc.gpsimd.collective_compute(
            "AllGather", mybir.AluOpType.bypass, replica_groups=RG,
            ins=[warm_in[:].opt()], outs=[warm_out[:].opt()])
        # preload the exp_and_others ACT table set (it also contains the
        # Copy/Relu fillers used earlier) so the first real exp in phase 3
        # doesn't pay the ~2.7us table load on the critical path
        exp_warm = s12.tile([1, 1], F32, name="exp_warm")
        nc.scalar.activation(exp_warm[:], warm_sb[0:1, 0:1], AF.Exp)

        # ---- XW1 (plain fp16) staged from host (queued before A so mm1
        # can start on the first A tile) ----
        xw1aug = s12.tile([P, KB, D_MID], F16, name="xw1aug")
        nc.sync.dma_start(
            xw1aug[:], xw1_in[:].rearrange("p (k c) -> p k c", c=D_MID))

        # ---- the A shard, resident in SBUF for both A-matmuls; its own
        # pool so phase 3's buffers can reuse the space as soon as mm2 ends
        apool_cm = tc.tile_pool(name="apool", bufs=1)
        apool = apool_cm.__enter__()
        a_tiles = []
        for kb in range(KB):
            at = apool.tile([P, SH], F16, name=f"a{kb}", tag=f"a{kb}")
            nc.sync.dma_start(at[:], a_t[kb * P:(kb + 1) * P, :])
            a_tiles.append(at)
            if kb == 40:
                # hold the last 23 A-tile issues (in-order Sync queue) until
                # the warm collective lands: the mesh barrier + warm AG
                # otherwise crawl under full HBM pressure (barrier-end tracks
                # A-load-end in every full-bore trace), serializing ~18us of
                # CC latency after mm1 instead of hiding it inside the load
                tier_gate = s12.tile([1, 16], F32, name="tier_gate")
                nc.sync.dma_start(tier_gate[:], warm_out[0, :, :])

        def a_tile(kb):
            return a_tiles[kb][:]

        hT = s12.tile([D_MID, SH], F32, name="hT")
        hw2T = s12.tile([D_EMB, SH], F32, name="hw2T")
        ag1_sb = s12.tile([P, 8 * D_EMB], F16, name="ag1_sb")

        # AG1 payload stays partition-major [128, 8*12] so every DMA on both
        # sides of the collective is contiguous (the nodes-major layout cost
        # ~18us of 20GB/s scatter/gather DMAs)
        ag1_in = dram.tile([P, 8 * D_EMB], F16, name="ag1_in")
        ag1_out = dram.tile([NCORES, P, 8 * D_EMB], F16, name="ag1_out")

        with tc.tile_pool(name="p1a", bufs=1, space="PSUM") as p1a:
            # mm1 n-half h: hT_aug[:, h*512:(h+1)*512]; as soon as half h of
            # hT is done, its hW2/transpose/AG1-half chain runs while the PE
            # streams the other half.
            hps = [p1a.tile([D_MID, 512], F32, name=f"hps{h}",
                            tag=f"hps{h}") for h in range(2)]
            h2ps = p1a.tile([D_EMB, SH], F32, name="h2ps")
            h2nat_ps = p1a.tile([P, 8 * D_EMB], F32, name="h2nat_ps")
            for kb in range(KB):
                for h in range(2):
                    nc.tensor.matmul(
                        hps[h][:], lhsT=xw1aug[:, kb, :],
                        rhs=a_tile(kb)[:, h * HS:(h + 1) * HS],
                        start=(kb == 0), stop=(kb == KB - 1),
                        skip_group_check=True)
            for h in range(2):
                sl = slice(h * HS, (h + 1) * HS)
                # hT half = relu(mm1 psum), one ACT op
                nc.scalar.activation(hT[:, sl], hps[h][:], AF.Relu)
                # hW2 half (exact f32) -> natural layout -> AG1 half
                nc.tensor.matmul(h2ps[:, sl], lhsT=w2_sb[:], rhs=hT[:, sl],
                                 start=True, stop=True)
                nc.scalar.copy(hw2T[:, sl], h2ps[:, sl])
                for qq in range(4):
                    q = h * 4 + qq
                    nc.tensor.transpose(
                        h2nat_ps[:, q * D_EMB:(q + 1) * D_EMB],
                        hw2T[:, q * P:(q + 1) * P],
                        ident[0:D_EMB, 0:D_EMB])
                qsl = slice(h * 4 * D_EMB, (h + 1) * 4 * D_EMB)
                nc.scalar.copy(ag1_sb[:, qsl], h2nat_ps[:, qsl])
                nc.sync.dma_start(ag1_in[:, qsl], ag1_sb[:, qsl])
            nc.gpsimd.collective_compute(
                "AllGather", mybir.AluOpType.bypass, replica_groups=RG,
                ins=[ag1_in[:].opt()], outs=[ag1_out[:].opt()])

        # ---- full hW2 (fp16) is mm2's stationary operand directly; rank r's
        # [128, 96] block lands at col r*96, so block kb sits at col kb*12 ----
        hw2n = s12.tile([P, KB * D_EMB], F16, name="hw2n")
        hw2n_f = hw2n[:].rearrange("p (k d) -> p k d", d=D_EMB)
        for r in range(NCORES):
            nc.sync.dma_start(
                hw2n[:, r * 8 * D_EMB:(r + 1) * 8 * D_EMB], ag1_out[r, :, :])

        # ---- mm2 + emb prep + AG2, in shard-row halves ----
        embT = s12.tile([D_EMB, SH], F32, name="embT")
        e_hi = s12.tile([D_EMB, SH], F16, name="e_hi")
        sqel = s12.tile([D_EMB, SH], F32, name="sqel")
        msq_hi = s12.tile([1, SH], F16, name="msq_hi")
        msq_lo = s12.tile([1, SH], F16, name="msq_lo")
        ag2_ins = [dram.tile([D_EMB + 2, HS], F16, name=f"ag2_in{h}")
                   for h in range(2)]
        ag2_outs = [dram.tile([NCORES, D_EMB + 2, HS], F16,
                              name=f"ag2_out{h}") for h in range(2)]
        neg_ones = s12.tile([D_EMB, 1], F32, name="neg_ones")
        nc.vector.memset(neg_ones[:], -1.0)
        # gram rhs lives on the right side, allocated before the mm2 loop so
        # each AG2 half's rhs columns load the moment that collective lands
        # (global col of (rank r, half hh, j) = r*1024 + hh*512 + j)
        s3a_cm = tc.tile_pool(name="s3a", bufs=1, side="right")
        s3a = s3a_cm.__enter__()
        rhs_sb = s3a.tile([KAUG, N], F16, name="rhs_sb")
        rhs_v = rhs_sb[:].rearrange("d (r hh j) -> d r hh j", r=NCORES, hh=2)
        s3a_box["cm"] = s3a_cm
        s3a_box["rhs"] = rhs_sb
        with tc.tile_pool(name="p2", bufs=1, space="PSUM") as p2:
            eps_ = [p2.tile([D_EMB, 512], F32, name=f"eps{h}",
                            tag=f"eps{h}") for h in range(2)]
            msq_ps = p2.tile([1, SH], F32, name="msq_ps")
            for h in range(2):
                sl = slice(h * HS, (h + 1) * HS)
                for kb in range(KB):
                    nc.tensor.matmul(
                        eps_[h][:], lhsT=hw2n_f[:, kb, :],
                        rhs=a_tile(kb)[:, sl],
                        start=(kb == 0), stop=(kb == KB - 1),
                        skip_group_check=True)
                nc.scalar.copy(embT[:, sl], eps_[h][:])
                # emb fp16 (hi only: the lo x hi Gram cross-terms are below
                # the error budget); -|emb|^2 stays hi/lo-split since sq
                # errors are column-systematic
                nc.scalar.copy(e_hi[:, sl], embT[:, sl])
                nc.scalar.mul(lhsT_sb[0:D_EMB, sl], e_hi[:, sl], 2.0)
                # -|emb_j|^2 via GPSIMD partition-reduce (no PSUM, so the
                # phase-3 PSUM pool isn't gated on this)
                nc.vector.tensor_mul(sqel[:, sl], embT[:, sl], embT[:, sl])
                nc.tensor.matmul(msq_ps[:, sl], lhsT=neg_ones[:],
                                 rhs=sqel[:, sl], start=True, stop=True)
                nc.scalar.copy(msq_hi[:, sl], msq_ps[:, sl])
                nc.vector.tensor_tensor(msq_lo[:, sl], msq_ps[:, sl],
                                        msq_hi[:, sl], SUB)
                # AG2 half: emb hi + (-sq) hi/lo, all fp16
                nc.sync.dma_start(ag2_ins[h][0:D_EMB, :], e_hi[:, sl])
                nc.sync.dma_start(ag2_ins[h][D_EMB:D_EMB + 1, :],
                                  msq_hi[:, sl])
                nc.sync.dma_start(ag2_ins[h][D_EMB + 1:D_EMB + 2, :],
                                  msq_lo[:, sl])
                nc.gpsimd.collective_compute(
                    "AllGather", mybir.AluOpType.bypass, replica_groups=RG,
                    ins=[ag2_ins[h][:].opt()], outs=[ag2_outs[h][:].opt()])
                for r in range(NCORES):
                    nc.sync.dma_start(rhs_v[:, r, h, :], ag2_outs[h][r, :, :])
                if h == 1:
                    apool_cm.__exit__(None, None, None)

    # ---- phase 3: logits -> exp -> row-normalize -> out ----
    rhs_sb = s3a_box["rhs"]
    with tc.tile_pool(name="s3", bufs=1) as s3, \
         tc.tile_pool(name="p3", bufs=1, space="PSUM") as p3:
        NMT = SH // P
        t_tiles, acc_tiles = {}, {}

        def mt_groups(mt, hh):
            # 2 PSUM groups of 4 chunks (ranks rg*4..rg*4+3) for half hh
            if mt not in t_tiles:
                t_tiles[mt] = s3.tile([P, N], BF16, name="t_sb", tag="t_sb",
                                      bufs=4)
                acc_tiles[mt] = s3.tile([P, 4], F32, name="acc", tag="acc",
                                        bufs=4)
            t_sb, acc = t_tiles[mt], acc_tiles[mt]
            t_v = t_sb[:].rearrange("p (r hh j) -> p r hh j",
                                    r=NCORES, hh=2)
            for rg in range(2):
                zps = p3.tile([P, 2048], F32, name="zps", tag="zps", bufs=2)
                for rr in range(4):
                    r = rg * 4 + rr
                    col = r * SH + hh * HS
                    nc.tensor.matmul(
                        zps[:, rr * 512:(rr + 1) * 512],
                        lhsT=lhsT_sb[:, mt * P:(mt + 1) * P],
                        rhs=rhs_sb[:, col:col + 512],
                        start=True, stop=True)
                nc.scalar.activation(
                    t_v[:, rg * 4:(rg + 1) * 4, hh, :],
                    zps[:].rearrange("p (r j) -> p r j", j=512), AF.Exp,
                    accum_out=acc[:, hh * 2 + rg:hh * 2 + rg + 1])

        def mt_finish(mt):
            t_sb, acc = t_tiles.pop(mt), acc_tiles.pop(mt)
            ssum = s3.tile([P, 1], F32, name="ssum", tag="ssum", bufs=4)
            nc.vector.reduce_sum(ssum[:], acc[:], axis=mybir.AxisListType.X)
            recip = s3.tile([P, 1], F32, name="recip", tag="recip", bufs=4)
            nc.vector.reciprocal(recip[:], ssum[:])
            for half in range(8):
                csl = slice(half * (N // 8), (half + 1) * (N // 8))
                nc.vector.tensor_scalar(t_sb[:, csl], t_sb[:, csl],
                                        recip[:], EPS, MULT, ADD)
                nc.sync.dma_start(out[mt * P:(mt + 1) * P, csl],
                                  t_sb[:, csl])

        # mt 0/1 run their a-half groups before any b-half group, so the
        # AG2b gather has slack to land without stalling ACT; after that the
        # schedule is mt-sequential so finished rows stream out early (the
        # 16.8MB output write is itself a ~55us stream that must start ASAP)
        mt_groups(0, 0)
        mt_groups(1, 0)
        mt_groups(0, 1)
        mt_finish(0)
        mt_groups(1, 1)
        mt_finish(1)
        for mt in range(2, NMT):
            mt_groups(mt, 0)
            mt_groups(mt, 1)
            mt_finish(mt)

    s3a_box["cm"].__exit__(None, None, None)
    sbp_cm.__exit__(None, None, None)
    dram_cm.__exit__(None, None, None)



def _build():
    nc = bacc.Bacc("TRN2", target_bir_lowering=False, debug=False,
                   num_devices=NCORES)
    a_t = nc.dram_tensor("a_t", [N, SH], F16, kind="ExternalInput")
    xw1_in = nc.dram_tensor("xw1_in", [P, KB * D_MID], F16,
                            kind="ExternalInput")
    w2 = nc.dram_tensor("w2", [D_MID, D_EMB], F32, kind="ExternalInput")
    out = nc.dram_tensor("out", [SH, N], BF16, kind="ExternalOutput")
    with tile.TileContext(nc) as tc:
        _emit(nc, tc, a_t.ap(), xw1_in.ap(), w2.ap(), out.ap())
    nc.compile()
    return nc


def _get_nc():
    global _NC
    if _NC is None:
        _NC = _build()
    return _NC


def _prep_in_maps(norm_adj_matrix, X, W1, W2):
    A = np.asarray(norm_adj_matrix, dtype=np.float32)
    X = np.asarray(X, dtype=np.float32)
    W1 = np.asarray(W1, dtype=np.float32)
    W2 = np.asarray(W2, dtype=np.float32)

    xw1 = X.astype(np.float64) @ W1.astype(np.float64)     # [N, 24]
    aug = xw1.astype(np.float16).reshape(KB, P, D_MID).transpose(1, 0, 2)
    xw1_in = np.ascontiguousarray(aug.reshape(P, KB * D_MID))

    in_maps = []
    for c in range(NCORES):
        a_t = A[c * SH:(c + 1) * SH, :].T.astype(np.float16)
        in_maps.append({"a_t": a_t, "xw1_in": xw1_in, "w2": W2})
    return in_maps


def _execute(in_maps, trace=False, tmpdir=None):
    # the first execution of a freshly loaded NEFF occasionally faults with
    # NRT_EXEC_UNIT_UNRECOVERABLE; a retry on a clean dispatch succeeds
    last = None
    for _ in range(3):
        try:
            return bass_utils.run_bass_kernel_spmd(
                _get_nc(), in_maps, core_ids=list(range(NCORES)),
                trace=trace, tmpdir=tmpdir)
        except Exception as e:  # noqa: BLE001
            last = e
    raise last


def _assemble(res):
    shards = [np.asarray(res.results[c]["out"]).astype(np.float32)
              for c in range(NCORES)]
    return np.concatenate(shards, axis=0)


def kernel(norm_adj_matrix, X, W1, W2):
    in_maps = _prep_in_maps(norm_adj_matrix, X, W1, W2)
    res = _execute(in_maps)
    return _assemble(res)



# revision 29
# speedup vs baseline: 1.1321x; 1.1321x over previous
"""AdaGAE forward on 8 TRN2 NeuronCores.

Computes, for A = norm_adj_matrix [8192, 8192]:
    h    = relu(A @ (X @ W1))           [n, 24]
    emb  = A @ (h @ W2)                 [n, 12]
    out  = softmax(-pairwise_sq_dists(emb), axis=1) + 1e-10

Key algebra: softmax is shift-invariant per row and the relu on squared
distances is a numerical no-op, so
    out[i, :] = softmax_j(2*<emb_i, emb_j> - |emb_j|^2) + eps
with the row-constant |emb_i|^2 cancelled. The -|emb_j|^2 bias is folded
into the Gram matmul as two extra contraction rows (ones x -sq_hi/lo), so
the whole logits matrix comes out of the PE in one 34-row fp16 matmul per
tile, straight into exp on the ScalarE with a free per-row accumulator,
then a fused multiply-add (x*recip + eps) normalize on DVE in bf16.

Distribution: row-shard A across 8 cores. Each core stages its shard as
A_shard.T in fp16 (fits SBUF entirely -> A is read from HBM exactly once),
computes hT/embT for its rows, and two tiny AllGathers (hW2, then the
emb+sq halves, pipelined against the second A-matmul and the Gram phase)
make the full embedding visible everywhere. fp16 is used for the big
streams (bf16 is too coarse for the softmax logits); XW1/hW2/emb are
plain fp16, -|emb|^2 ships hi+lo fp16 (its error is column-systematic),
and everything accumulates in f32 PSUM. ~210 us on silicon, L2 err 2.9e-3.
"""
import sys

for _p in ("/opt/trn_rl_repo", "/opt/trn_rl_repo/concourse"):
    if _p not in sys.path:
        sys.path.insert(0, _p)

import numpy as np

from concourse import bacc, mybir, tile
from concourse import bass_utils
from concourse.masks import make_identity

F32 = mybir.dt.float32
F16 = mybir.dt.float16
BF16 = mybir.dt.bfloat16
ADD = mybir.AluOpType.add
SUB = mybir.AluOpType.subtract
MULT = mybir.AluOpType.mult
AF = mybir.ActivationFunctionType

N = 8192
NCORES = 8
SH = N // NCORES          # 1024 rows per core
P = 128
KB = N // P               # 64 contraction blocks
D_IN, D_MID, D_EMB = 32, 24, 12
KAUG = 14                 # rows 0:12 = 2e_hi | e_hi, rows 12:14 = ones | -sq_hi/lo
EPS = 1e-10
RG = [list(range(NCORES))]

_NC = None


def _emit(nc, tc, a_t, xw1_in, w2, out):
    HS = SH // 2              # 512: shard-row half for collective overlap
    dram_cm = tc.tile_pool(name="dram", bufs=1, space="DRAM")
    dram = dram_cm.__enter__()
    sbp_cm = tc.tile_pool(name="sbp", bufs=1)
    sbp = sbp_cm.__enter__()

    # persists into phase 3: the Gram matmul's stationary operand. Engine
    # writes must start at a partition multiple of 32, so the ones rows
    # (12:14) are set by memsetting all 14 rows; rows 0:12 are overwritten
    # with 2*e_hi in phase 2.
    lhsT_sb = sbp.tile([KAUG, SH], F16, name="lhsT_sb")
    nc.vector.memset(lhsT_sb[:], 1.0)
    s3a_box = {}

    with tc.tile_pool(name="s12", bufs=1) as s12:
        # ---- small inputs ----
        w2_sb = s12.tile([D_MID, D_EMB], F32, name="w2_sb")
        nc.sync.dma_start(w2_sb[:], w2[:])
        ident = s12.tile([P, P], F32, name="ident")
        make_identity(nc, ident)

        # tiny first collective: the first AllGather on the CC stream pays
        # ~2x duration (NRT first-call staging), so burn that cost on a 64B
        # op during the A-load window instead of on AG1
        warm_sb = s12.tile([1, 16], F32, name="warm_sb")
        nc.vector.memset(warm_sb[:], 0.0)
        warm_in = dram.tile([1, 16], F32, name="warm_in")
        nc.sync.dma_start(warm_in[:], warm_sb[:])
        warm_out = dram.tile([NCORES, 1, 16], F32, name="warm_out")
        nc.gpsimd.collective_compute(
            "AllGather", mybir.AluOpType.bypass, replica_groups=RG,
            ins=[warm_in[:].opt()], outs=[warm_out[:].opt()])
        # preload the exp_and_others ACT table set (it also contains the
        # Copy/Relu fillers used earlier) so the first real exp in phase 3
        # doesn't pay the ~2.7us table load on the critical path
        exp_warm = s12.tile([1, 1], F32, name="exp_warm")
        nc.scalar.activation(exp_warm[:], warm_sb[0:1, 0:1], AF.Exp)

        # ---- XW1 (plain fp16) staged from host (queued before A so mm1
        # can start on the first A tile) ----
        xw1aug = s12.tile([P, KB, D_MID], F16, name="xw1aug")
        nc.sync.dma_start(
            xw1aug[:], xw1_in[:].rearrange("p (k c) -> p k c", c=D_MID))

        # ---- the A shard, resident in SBUF for both A-matmuls; its own
        # pool so phase 3's buffers can reuse the space as soon as mm2 ends
        apool_cm = tc.tile_pool(name="apool", bufs=1)
        apool = apool_cm.__enter__()
        a_tiles = []
        for kb in range(KB):
            at = apool.tile([P, SH], F16, name=f"a{kb}", tag=f"a{kb}")
            nc.sync.dma_start(at[:], a_t[kb * P:(kb + 1) * P, :])
            a_tiles.append(at)

        def a_tile(kb):
            return a_tiles[kb][:]

        hT = s12.tile([D_MID, SH], F32, name="hT")
        hw2T = s12.tile([D_EMB, SH], F32, name="hw2T")
        ag1_sb = s12.tile([P, 8 * D_EMB], F16, name="ag1_sb")

        # AG1 payload stays partition-major [128, 8*12] so every DMA on both
        # sides of the collective is contiguous (the nodes-major layout cost
        # ~18us of 20GB/s scatter/gather DMAs)
        ag1_in = dram.tile([P, 8 * D_EMB], F16, name="ag1_in")
        ag1_out = dram.tile([NCORES, P, 8 * D_EMB], F16, name="ag1_out")

        with tc.tile_pool(name="p1a", bufs=1, space="PSUM") as p1a:
            # mm1 n-half h: hT_aug[:, h*512:(h+1)*512]; as soon as half h of
            # hT is done, its hW2/transpose/AG1-half chain runs while the PE
            # streams the other half.
            hps = [p1a.tile([D_MID, 512], F32, name=f"hps{h}",
                            tag=f"hps{h}") for h in range(2)]
            h2ps = p1a.tile([D_EMB, SH], F32, name="h2ps")
            h2nat_ps = p1a.tile([P, 8 * D_EMB], F32, name="h2nat_ps")
            for kb in range(KB):
                for h in range(2):
                    nc.tensor.matmul(
                        hps[h][:], lhsT=xw1aug[:, kb, :],
                        rhs=a_tile(kb)[:, h * HS:(h + 1) * HS],
                        start=(kb == 0), stop=(kb == KB - 1),
                        skip_group_check=True)
            for h in range(2):
                sl = slice(h * HS, (h + 1) * HS)
                # hT half = relu(mm1 psum), one ACT op
                nc.scalar.activation(hT[:, sl], hps[h][:], AF.Relu)
                # hW2 half (exact f32) -> natural layout -> AG1 half
                nc.tensor.matmul(h2ps[:, sl], lhsT=w2_sb[:], rhs=hT[:, sl],
                                 start=True, stop=True)
                nc.scalar.copy(hw2T[:, sl], h2ps[:, sl])
                for qq in range(4):
                    q = h * 4 + qq
                    nc.tensor.transpose(
                        h2nat_ps[:, q * D_EMB:(q + 1) * D_EMB],
                        hw2T[:, q * P:(q + 1) * P],
                        ident[0:D_EMB, 0:D_EMB])
                qsl = slice(h * 4 * D_EMB, (h + 1) * 4 * D_EMB)
                nc.scalar.copy(ag1_sb[:, qsl], h2nat_ps[:, qsl])
                nc.sync.dma_start(ag1_in[:, qsl], ag1_sb[:, qsl])
            nc.gpsimd.collective_compute(
                "AllGather", mybir.AluOpType.bypass, replica_groups=RG,
                ins=[ag1_in[:].opt()], outs=[ag1_out[:].opt()])

        # ---- full hW2 (fp16) is mm2's stationary operand directly; rank r's
        # [128, 96] block lands at col r*96, so block kb sits at col kb*12 ----
        hw2n = s12.tile([P, KB * D_EMB], F16, name="hw2n")
        hw2n_f = hw2n[:].rearrange("p (k d) -> p k d", d=D_EMB)
        for r in range(NCORES):
            nc.sync.dma_start(
                hw2n[:, r * 8 * D_EMB:(r + 1) * 8 * D_EMB], ag1_out[r, :, :])

        # ---- mm2 + emb prep + AG2, in shard-row halves ----
        e_hi = s12.tile([D_EMB, SH], F16, name="e_hi")
        sqel = s12.tile([D_EMB, SH], F32, name="sqel")
        msq_hi = s12.tile([1, SH], F16, name="msq_hi")
        msq_lo = s12.tile([1, SH], F16, name="msq_lo")
        ag2_ins = [dram.tile([D_EMB + 2, HS], F16, name=f"ag2_in{h}")
                   for h in range(2)]
        ag2_outs = [dram.tile([NCORES, D_EMB + 2, HS], F16,
                              name=f"ag2_out{h}") for h in range(2)]
        neg_ones = s12.tile([D_EMB, 1], F32, name="neg_ones")
        nc.vector.memset(neg_ones[:], -1.0)
        # gram rhs lives on the right side, allocated before the mm2 loop so
        # each AG2 half's rhs columns load the moment that collective lands
        # (global col of (rank r, half hh, j) = r*1024 + hh*512 + j)
        s3a_cm = tc.tile_pool(name="s3a", bufs=1, side="right")
        s3a = s3a_cm.__enter__()
        rhs_sb = s3a.tile([KAUG, N], F16, name="rhs_sb")
        rhs_v = rhs_sb[:].rearrange("d (r hh j) -> d r hh j", r=NCORES, hh=2)
        s3a_box["cm"] = s3a_cm
        s3a_box["rhs"] = rhs_sb
        with tc.tile_pool(name="p2", bufs=1, space="PSUM") as p2:
            eps_ = [p2.tile([D_EMB, 512], F32, name=f"eps{h}",
                            tag=f"eps{h}") for h in range(2)]
            msq_ps = p2.tile([1, SH], F32, name="msq_ps")
            for h in range(2):
                sl = slice(h * HS, (h + 1) * HS)
                for kb in range(KB):
                    nc.tensor.matmul(
                        eps_[h][:], lhsT=hw2n_f[:, kb, :],
                        rhs=a_tile(kb)[:, sl],
                        start=(kb == 0), stop=(kb == KB - 1),
                        skip_group_check=True)
                # emb fp16 (hi only: the lo x hi Gram cross-terms are below
                # the error budget); -|emb|^2 stays hi/lo-split since sq
                # errors are column-systematic. All three consumers read the
                # mm2 PSUM directly -- no f32 embT staging hop on the
                # mm2-end -> AG2 serial chain.
                nc.scalar.copy(e_hi[:, sl], eps_[h][:])
                nc.scalar.mul(lhsT_sb[0:D_EMB, sl], eps_[h][:], 2.0)
                # one PSUM operand max per DVE op; e_hi*emb ~= emb^2 to
                # ~3e-4 rel, below the fp16 msq quantization already shipped
                nc.vector.tensor_mul(sqel[:, sl], e_hi[:, sl], eps_[h][:])
                nc.tensor.matmul(msq_ps[:, sl], lhsT=neg_ones[:],
                                 rhs=sqel[:, sl], start=True, stop=True)
                nc.scalar.copy(msq_hi[:, sl], msq_ps[:, sl])
                nc.vector.tensor_tensor(msq_lo[:, sl], msq_ps[:, sl],
                                        msq_hi[:, sl], SUB)
                # AG2 half: emb hi + (-sq) hi/lo, all fp16
                nc.sync.dma_start(ag2_ins[h][0:D_EMB, :], e_hi[:, sl])
                nc.sync.dma_start(ag2_ins[h][D_EMB:D_EMB + 1, :],
                                  msq_hi[:, sl])
                nc.sync.dma_start(ag2_ins[h][D_EMB + 1:D_EMB + 2, :],
                                  msq_lo[:, sl])
                nc.gpsimd.collective_compute(
                    "AllGather", mybir.AluOpType.bypass, replica_groups=RG,
                    ins=[ag2_ins[h][:].opt()], outs=[ag2_outs[h][:].opt()])
                for r in range(NCORES):
                    nc.sync.dma_start(rhs_v[:, r, h, :], ag2_outs[h][r, :, :])
                if h == 1:
                    apool_cm.__exit__(None, None, None)

    # ---- phase 3: logits -> exp -> row-normalize -> out ----
    rhs_sb = s3a_box["rhs"]
    with tc.tile_pool(name="s3", bufs=1) as s3, \
         tc.tile_pool(name="p3", bufs=1, space="PSUM") as p3:
        NMT = SH // P
        t_tiles, acc_tiles = {}, {}

        def mt_groups(mt, hh):
            # 2 PSUM groups of 4 chunks (ranks rg*4..rg*4+3) for half hh
            if mt not in t_tiles:
                t_tiles[mt] = s3.tile([P, N], BF16, name="t_sb", tag="t_sb",
                                      bufs=4)
                acc_tiles[mt] = s3.tile([P, 4], F32, name="acc", tag="acc",
                                        bufs=4)
            t_sb, acc = t_tiles[mt], acc_tiles[mt]
            t_v = t_sb[:].rearrange("p (r hh j) -> p r hh j",
                                    r=NCORES, hh=2)
            for rg in range(2):
                zps = p3.tile([P, 2048], F32, name="zps", tag="zps", bufs=2)
                for rr in range(4):
                    r = rg * 4 + rr
                    col = r * SH + hh * HS
                    nc.tensor.matmul(
                        zps[:, rr * 512:(rr + 1) * 512],
                        lhsT=lhsT_sb[:, mt * P:(mt + 1) * P],
                        rhs=rhs_sb[:, col:col + 512],
                        start=True, stop=True)
                nc.scalar.activation(
                    t_v[:, rg * 4:(rg + 1) * 4, hh, :],
                    zps[:].rearrange("p (r j) -> p r j", j=512), AF.Exp,
                    accum_out=acc[:, hh * 2 + rg:hh * 2 + rg + 1])

        def mt_finish(mt):
            t_sb, acc = t_tiles.pop(mt), acc_tiles.pop(mt)
            ssum = s3.tile([P, 1], F32, name="ssum", tag="ssum", bufs=4)
            nc.vector.reduce_sum(ssum[:], acc[:], axis=mybir.AxisListType.X)
            recip = s3.tile([P, 1], F32, name="recip", tag="recip", bufs=4)
            nc.vector.reciprocal(recip[:], ssum[:])
            for half in range(8):
                csl = slice(half * (N // 8), (half + 1) * (N // 8))
                nc.vector.tensor_scalar(t_sb[:, csl], t_sb[:, csl],
                                        recip[:], EPS, MULT, ADD)
                nc.sync.dma_start(out[mt * P:(mt + 1) * P, csl],
                                  t_sb[:, csl])

        # mt 0/1 run their a-half groups before any b-half group, so the
        # AG2b gather has slack to land without stalling ACT; after that the
        # schedule is mt-sequential so finished rows stream out early (the
        # 16.8MB output write is itself a ~55us stream that must start ASAP)
        mt_groups(0, 0)
        mt_groups(1, 0)
        mt_groups(0, 1)
        mt_finish(0)
        mt_groups(1, 1)
        mt_finish(1)
        for mt in range(2, NMT):
            mt_groups(mt, 0)
            mt_groups(mt, 1)
            mt_finish(mt)

    s3a_box["cm"].__exit__(None, None, None)
    sbp_cm.__exit__(None, None, None)
    dram_cm.__exit__(None, None, None)



def _build():
    nc = bacc.Bacc("TRN2", target_bir_lowering=False, debug=False,
                   num_devices=NCORES)
    a_t = nc.dram_tensor("a_t", [N, SH], F16, kind="ExternalInput")
    xw1_in = nc.dram_tensor("xw1_in", [P, KB * D_MID], F16,
                            kind="ExternalInput")
    w2 = nc.dram_tensor("w2", [D_MID, D_EMB], F32, kind="ExternalInput")
    out = nc.dram_tensor("out", [SH, N], BF16, kind="ExternalOutput")
    with tile.TileContext(nc) as tc:
        _emit(nc, tc, a_t.ap(), xw1_in.ap(), w2.ap(), out.ap())
    nc.compile()
    return nc


def _get_nc():
    global _NC
    if _NC is None:
        _NC = _build()
    return _NC


def _prep_in_maps(norm_adj_matrix, X, W1, W2):
    A = np.asarray(norm_adj_matrix, dtype=np.float32)
    X = np.asarray(X, dtype=np.float32)
    W1 = np.asarray(W1, dtype=np.float32)
    W2 = np.asarray(W2, dtype=np.float32)

    xw1 = X.astype(np.float64) @ W1.astype(np.float64)     # [N, 24]
    aug = xw1.astype(np.float16).reshape(KB, P, D_MID).transpose(1, 0, 2)
    xw1_in = np.ascontiguousarray(aug.reshape(P, KB * D_MID))

    in_maps = []
    for c in range(NCORES):
        a_t = A[c * SH:(c + 1) * SH, :].T.astype(np.float16)
        in_maps.append({"a_t": a_t, "xw1_in": xw1_in, "w2": W2})
    return in_maps


def _execute(in_maps, trace=False, tmpdir=None):
    # the first execution of a freshly loaded NEFF occasionally faults with
    # NRT_EXEC_UNIT_UNRECOVERABLE; a retry on a clean dispatch succeeds
    last = None
    for _ in range(3):
        try:
            return bass_utils.run_bass_kernel_spmd(
                _get_nc(), in_maps, core_ids=list(range(NCORES)),
                trace=trace, tmpdir=tmpdir)
        except Exception as e:  # noqa: BLE001
            last = e
    raise last


def _assemble(res):
    shards = [np.asarray(res.results[c]["out"]).astype(np.float32)
              for c in range(NCORES)]
    return np.concatenate(shards, axis=0)


def kernel(norm_adj_matrix, X, W1, W2):
    in_maps = _prep_in_maps(norm_adj_matrix, X, W1, W2)
    res = _execute(in_maps)
    return _assemble(res)



# revision 32
# speedup vs baseline: 1.1448x; 1.0113x over previous
"""AdaGAE forward on 8 TRN2 NeuronCores.

Computes, for A = norm_adj_matrix [8192, 8192]:
    h    = relu(A @ (X @ W1))           [n, 24]
    emb  = A @ (h @ W2)                 [n, 12]
    out  = softmax(-pairwise_sq_dists(emb), axis=1) + 1e-10

Key algebra: softmax is shift-invariant per row and the relu on squared
distances is a numerical no-op, so
    out[i, :] = softmax_j(2*<emb_i, emb_j> - |emb_j|^2) + eps
with the row-constant |emb_i|^2 cancelled. The -|emb_j|^2 bias is folded
into the Gram matmul as two extra contraction rows (ones x -sq_hi/lo), so
the whole logits matrix comes out of the PE in one 34-row fp16 matmul per
tile, straight into exp on the ScalarE with a free per-row accumulator,
then a fused multiply-add (x*recip + eps) normalize on DVE in bf16.

Distribution: row-shard A across 8 cores. Each core stages its shard as
A_shard.T in fp16 (fits SBUF entirely -> A is read from HBM exactly once),
computes hT/embT for its rows, and two tiny AllGathers (hW2, then the
emb+sq halves, pipelined against the second A-matmul and the Gram phase)
make the full embedding visible everywhere. fp16 is used for the big
streams (bf16 is too coarse for the softmax logits); XW1/hW2/emb are
plain fp16, -|emb|^2 ships hi+lo fp16 (its error is column-systematic),
and everything accumulates in f32 PSUM. ~210 us on silicon, L2 err 2.9e-3.
"""
import sys

for _p in ("/opt/trn_rl_repo", "/opt/trn_rl_repo/concourse"):
    if _p not in sys.path:
        sys.path.insert(0, _p)

import numpy as np

from concourse import bacc, mybir, tile
from concourse import bass_utils
from concourse.masks import make_identity

F32 = mybir.dt.float32
F16 = mybir.dt.float16
BF16 = mybir.dt.bfloat16
ADD = mybir.AluOpType.add
SUB = mybir.AluOpType.subtract
MULT = mybir.AluOpType.mult
AF = mybir.ActivationFunctionType

N = 8192
NCORES = 8
SH = N // NCORES          # 1024 rows per core
P = 128
KB = N // P               # 64 contraction blocks
D_IN, D_MID, D_EMB = 32, 24, 12
KAUG = 14                 # rows 0:12 = 2e_hi | e_hi, rows 12:14 = ones | -sq_hi/lo
EPS = 1e-10
RG = [list(range(NCORES))]

_NC = None


def _emit(nc, tc, a_t, xw1_in, w2, out):
    HS = SH // 2              # 512: shard-row half for collective overlap
    dram_cm = tc.tile_pool(name="dram", bufs=1, space="DRAM")
    dram = dram_cm.__enter__()
    sbp_cm = tc.tile_pool(name="sbp", bufs=1)
    sbp = sbp_cm.__enter__()

    # persists into phase 3: the Gram matmul's stationary operand. Engine
    # writes must start at a partition multiple of 32, so the ones rows
    # (12:14) are set by memsetting all 14 rows; rows 0:12 are overwritten
    # with 2*e_hi in phase 2.
    lhsT_sb = sbp.tile([KAUG, SH], F16, name="lhsT_sb")
    nc.vector.memset(lhsT_sb[:], 1.0)
    s3a_box = {}

    with tc.tile_pool(name="s12", bufs=1) as s12:
        # ---- small inputs ----
        w2_sb = s12.tile([D_MID, D_EMB], F32, name="w2_sb")
        nc.sync.dma_start(w2_sb[:], w2[:])
        ident = s12.tile([P, P], F32, name="ident")
        make_identity(nc, ident)

        # tiny first collective: the first AllGather on the CC stream pays
        # ~2x duration (NRT first-call staging), so burn that cost on a 64B
        # op during the A-load window instead of on AG1
        warm_sb = s12.tile([1, 16], F32, name="warm_sb")
        nc.vector.memset(warm_sb[:], 0.0)
        warm_in = dram.tile([1, 16], F32, name="warm_in")
        nc.sync.dma_start(warm_in[:], warm_sb[:])
        warm_out = dram.tile([NCORES, 1, 16], F32, name="warm_out")
        nc.gpsimd.collective_compute(
            "AllGather", mybir.AluOpType.bypass, replica_groups=RG,
            ins=[warm_in[:].opt()], outs=[warm_out[:].opt()])
        # preload the exp_and_others ACT table set (it also contains the
        # Copy/Relu fillers used earlier) so the first real exp in phase 3
        # doesn't pay the ~2.7us table load on the critical path
        exp_warm = s12.tile([1, 1], F32, name="exp_warm")
        nc.scalar.activation(exp_warm[:], warm_sb[0:1, 0:1], AF.Exp)

        # ---- XW1 (plain fp16) staged from host (queued before A so mm1
        # can start on the first A tile) ----
        xw1aug = s12.tile([P, KB, D_MID], F16, name="xw1aug")
        nc.sync.dma_start(
            xw1aug[:], xw1_in[:].rearrange("p (k c) -> p k c", c=D_MID))

        # ---- the A shard, resident in SBUF for both A-matmuls; its own
        # pool so phase 3's buffers can reuse the space as soon as mm2 ends
        apool_cm = tc.tile_pool(name="apool", bufs=1)
        apool = apool_cm.__enter__()
        # 8 DMAs of 2MB with 16KB-contiguous partition lines (host pre-packs
        # a_t so group g's 8 kb-blocks lie side by side per partition row) --
        # the 64x256KB/2KB-line variant left ~10us of DMA ramp/issue slop
        a_tiles = []
        for g in range(8):
            at = apool.tile([P, 8 * SH], F16, name=f"a{g}", tag=f"a{g}")
            nc.sync.dma_start(at[:], a_t[g, :, :])
            a_tiles.append(at)

        def a_tile(kb):
            return a_tiles[kb // 8][:, (kb % 8) * SH:(kb % 8 + 1) * SH]

        hT = s12.tile([D_MID, SH], F32, name="hT")
        hw2T = s12.tile([D_EMB, SH], F32, name="hw2T")
        ag1_sb = s12.tile([P, 8 * D_EMB], F16, name="ag1_sb")

        # AG1 payload stays partition-major [128, 8*12] so every DMA on both
        # sides of the collective is contiguous (the nodes-major layout cost
        # ~18us of 20GB/s scatter/gather DMAs)
        ag1_in = dram.tile([P, 8 * D_EMB], F16, name="ag1_in")
        ag1_out = dram.tile([NCORES, P, 8 * D_EMB], F16, name="ag1_out")

        with tc.tile_pool(name="p1a", bufs=1, space="PSUM") as p1a:
            # mm1 n-half h: hT_aug[:, h*512:(h+1)*512]; as soon as half h of
            # hT is done, its hW2/transpose/AG1-half chain runs while the PE
            # streams the other half.
            hps = [p1a.tile([D_MID, 512], F32, name=f"hps{h}",
                            tag=f"hps{h}") for h in range(2)]
            h2ps = p1a.tile([D_EMB, SH], F32, name="h2ps")
            h2nat_ps = p1a.tile([P, 8 * D_EMB], F32, name="h2nat_ps")
            for kb in range(KB):
                for h in range(2):
                    nc.tensor.matmul(
                        hps[h][:], lhsT=xw1aug[:, kb, :],
                        rhs=a_tile(kb)[:, h * HS:(h + 1) * HS],
                        start=(kb == 0), stop=(kb == KB - 1),
                        skip_group_check=True)
            for h in range(2):
                sl = slice(h * HS, (h + 1) * HS)
                # hT half = relu(mm1 psum), one ACT op
                nc.scalar.activation(hT[:, sl], hps[h][:], AF.Relu)
                # hW2 half (exact f32) -> natural layout -> AG1 half
                nc.tensor.matmul(h2ps[:, sl], lhsT=w2_sb[:], rhs=hT[:, sl],
                                 start=True, stop=True)
                nc.scalar.copy(hw2T[:, sl], h2ps[:, sl])
                for qq in range(4):
                    q = h * 4 + qq
                    nc.tensor.transpose(
                        h2nat_ps[:, q * D_EMB:(q + 1) * D_EMB],
                        hw2T[:, q * P:(q + 1) * P],
                        ident[0:D_EMB, 0:D_EMB])
                qsl = slice(h * 4 * D_EMB, (h + 1) * 4 * D_EMB)
                nc.scalar.copy(ag1_sb[:, qsl], h2nat_ps[:, qsl])
                nc.sync.dma_start(ag1_in[:, qsl], ag1_sb[:, qsl])
            nc.gpsimd.collective_compute(
                "AllGather", mybir.AluOpType.bypass, replica_groups=RG,
                ins=[ag1_in[:].opt()], outs=[ag1_out[:].opt()])

        # ---- full hW2 (fp16) is mm2's stationary operand directly; rank r's
        # [128, 96] block lands at col r*96, so block kb sits at col kb*12 ----
        hw2n = s12.tile([P, KB * D_EMB], F16, name="hw2n")
        hw2n_f = hw2n[:].rearrange("p (k d) -> p k d", d=D_EMB)
        for r in range(NCORES):
            nc.sync.dma_start(
                hw2n[:, r * 8 * D_EMB:(r + 1) * 8 * D_EMB], ag1_out[r, :, :])

        # ---- mm2 + emb prep + AG2, in shard-row halves ----
        e_hi = s12.tile([D_EMB, SH], F16, name="e_hi")
        sqel = s12.tile([D_EMB, SH], F32, name="sqel")
        msq_hi = s12.tile([1, SH], F16, name="msq_hi")
        msq_lo = s12.tile([1, SH], F16, name="msq_lo")
        ag2_ins = [dram.tile([D_EMB + 2, HS], F16, name=f"ag2_in{h}")
                   for h in range(2)]
        ag2_outs = [dram.tile([NCORES, D_EMB + 2, HS], F16,
                              name=f"ag2_out{h}") for h in range(2)]
        neg_ones = s12.tile([D_EMB, 1], F32, name="neg_ones")
        nc.vector.memset(neg_ones[:], -1.0)
        # gram rhs lives on the right side, allocated before the mm2 loop so
        # each AG2 half's rhs columns load the moment that collective lands
        # (global col of (rank r, half hh, j) = r*1024 + hh*512 + j)
        s3a_cm = tc.tile_pool(name="s3a", bufs=1, side="right")
        s3a = s3a_cm.__enter__()
        rhs_sb = s3a.tile([KAUG, N], F16, name="rhs_sb")
        rhs_v = rhs_sb[:].rearrange("d (r hh j) -> d r hh j", r=NCORES, hh=2)
        s3a_box["cm"] = s3a_cm
        s3a_box["rhs"] = rhs_sb
        with tc.tile_pool(name="p2", bufs=1, space="PSUM") as p2:
            eps_ = [p2.tile([D_EMB, 512], F32, name=f"eps{h}",
                            tag=f"eps{h}") for h in range(2)]
            msq_ps = p2.tile([1, SH], F32, name="msq_ps")
            for h in range(2):
                sl = slice(h * HS, (h + 1) * HS)
                for kb in range(KB):
                    nc.tensor.matmul(
                        eps_[h][:], lhsT=hw2n_f[:, kb, :],
                        rhs=a_tile(kb)[:, sl],
                        start=(kb == 0), stop=(kb == KB - 1),
                        skip_group_check=True)
                # emb fp16 (hi only: the lo x hi Gram cross-terms are below
                # the error budget); -|emb|^2 stays hi/lo-split since sq
                # errors are column-systematic. All three consumers read the
                # mm2 PSUM directly -- no f32 embT staging hop on the
                # mm2-end -> AG2 serial chain.
                nc.scalar.copy(e_hi[:, sl], eps_[h][:])
                nc.scalar.mul(lhsT_sb[0:D_EMB, sl], eps_[h][:], 2.0)
                # one PSUM operand max per DVE op; e_hi*emb ~= emb^2 to
                # ~3e-4 rel, below the fp16 msq quantization already shipped
                nc.vector.tensor_mul(sqel[:, sl], e_hi[:, sl], eps_[h][:])
                nc.tensor.matmul(msq_ps[:, sl], lhsT=neg_ones[:],
                                 rhs=sqel[:, sl], start=True, stop=True)
                nc.scalar.copy(msq_hi[:, sl], msq_ps[:, sl])
                nc.vector.tensor_tensor(msq_lo[:, sl], msq_ps[:, sl],
                                        msq_hi[:, sl], SUB)
                # AG2 half: emb hi + (-sq) hi/lo, all fp16
                nc.sync.dma_start(ag2_ins[h][0:D_EMB, :], e_hi[:, sl])
                nc.sync.dma_start(ag2_ins[h][D_EMB:D_EMB + 1, :],
                                  msq_hi[:, sl])
                nc.sync.dma_start(ag2_ins[h][D_EMB + 1:D_EMB + 2, :],
                                  msq_lo[:, sl])
                nc.gpsimd.collective_compute(
                    "AllGather", mybir.AluOpType.bypass, replica_groups=RG,
                    ins=[ag2_ins[h][:].opt()], outs=[ag2_outs[h][:].opt()])
                for r in range(NCORES):
                    nc.sync.dma_start(rhs_v[:, r, h, :], ag2_outs[h][r, :, :])
                if h == 1:
                    apool_cm.__exit__(None, None, None)

    # ---- phase 3: logits -> exp -> row-normalize -> out ----
    rhs_sb = s3a_box["rhs"]
    with tc.tile_pool(name="s3", bufs=1) as s3, \
         tc.tile_pool(name="p3", bufs=1, space="PSUM") as p3:
        NMT = SH // P
        t_tiles, acc_tiles = {}, {}

        def mt_groups(mt, hh):
            # 2 PSUM groups of 4 chunks (ranks rg*4..rg*4+3) for half hh
            if mt not in t_tiles:
                t_tiles[mt] = s3.tile([P, N], BF16, name="t_sb", tag="t_sb",
                                      bufs=4)
                acc_tiles[mt] = s3.tile([P, 4], F32, name="acc", tag="acc",
                                        bufs=4)
            t_sb, acc = t_tiles[mt], acc_tiles[mt]
            t_v = t_sb[:].rearrange("p (r hh j) -> p r hh j",
                                    r=NCORES, hh=2)
            for rg in range(2):
                zps = p3.tile([P, 2048], F32, name="zps", tag="zps", bufs=2)
                for rr in range(4):
                    r = rg * 4 + rr
                    col = r * SH + hh * HS
                    nc.tensor.matmul(
                        zps[:, rr * 512:(rr + 1) * 512],
                        lhsT=lhsT_sb[:, mt * P:(mt + 1) * P],
                        rhs=rhs_sb[:, col:col + 512],
                        start=True, stop=True)
                nc.scalar.activation(
                    t_v[:, rg * 4:(rg + 1) * 4, hh, :],
                    zps[:].rearrange("p (r j) -> p r j", j=512), AF.Exp,
                    accum_out=acc[:, hh * 2 + rg:hh * 2 + rg + 1])

        def mt_finish(mt):
            t_sb, acc = t_tiles.pop(mt), acc_tiles.pop(mt)
            ssum = s3.tile([P, 1], F32, name="ssum", tag="ssum", bufs=4)
            nc.vector.reduce_sum(ssum[:], acc[:], axis=mybir.AxisListType.X)
            recip = s3.tile([P, 1], F32, name="recip", tag="recip", bufs=4)
            nc.vector.reciprocal(recip[:], ssum[:])
            for half in range(8):
                csl = slice(half * (N // 8), (half + 1) * (N // 8))
                nc.vector.tensor_scalar(t_sb[:, csl], t_sb[:, csl],
                                        recip[:], EPS, MULT, ADD)
                nc.sync.dma_start(out[mt * P:(mt + 1) * P, csl],
                                  t_sb[:, csl])

        # mt 0/1 run their a-half groups before any b-half group, so the
        # AG2b gather has slack to land without stalling ACT; after that the
        # schedule is mt-sequential so finished rows stream out early (the
        # 16.8MB output write is itself a ~55us stream that must start ASAP)
        mt_groups(0, 0)
        mt_groups(1, 0)
        mt_groups(0, 1)
        mt_finish(0)
        mt_groups(1, 1)
        mt_finish(1)
        for mt in range(2, NMT):
            mt_groups(mt, 0)
            mt_groups(mt, 1)
            mt_finish(mt)

    s3a_box["cm"].__exit__(None, None, None)
    sbp_cm.__exit__(None, None, None)
    dram_cm.__exit__(None, None, None)



def _build():
    nc = bacc.Bacc("TRN2", target_bir_lowering=False, debug=False,
                   num_devices=NCORES)
    a_t = nc.dram_tensor("a_t", [8, P, 8 * SH], F16, kind="ExternalInput")
    xw1_in = nc.dram_tensor("xw1_in", [P, KB * D_MID], F16,
                            kind="ExternalInput")
    w2 = nc.dram_tensor("w2", [D_MID, D_EMB], F32, kind="ExternalInput")
    out = nc.dram_tensor("out", [SH, N], BF16, kind="ExternalOutput")
    with tile.TileContext(nc) as tc:
        _emit(nc, tc, a_t.ap(), xw1_in.ap(), w2.ap(), out.ap())
    nc.compile()
    return nc


def _get_nc():
    global _NC
    if _NC is None:
        _NC = _build()
    return _NC


def _prep_in_maps(norm_adj_matrix, X, W1, W2):
    A = np.asarray(norm_adj_matrix, dtype=np.float32)
    X = np.asarray(X, dtype=np.float32)
    W1 = np.asarray(W1, dtype=np.float32)
    W2 = np.asarray(W2, dtype=np.float32)

    xw1 = X.astype(np.float64) @ W1.astype(np.float64)     # [N, 24]
    aug = xw1.astype(np.float16).reshape(KB, P, D_MID).transpose(1, 0, 2)
    xw1_in = np.ascontiguousarray(aug.reshape(P, KB * D_MID))

    in_maps = []
    for c in range(NCORES):
        at = A[c * SH:(c + 1) * SH, :].T.astype(np.float16)   # [8192, 1024]
        # pack so each of 8 big device DMAs reads 16KB contiguous per
        # partition row: a_t[g, p, kk*1024 + j] = at[(g*8 + kk)*128 + p, j]
        a_t = np.ascontiguousarray(
            at.reshape(8, 8, P, SH).transpose(0, 2, 1, 3).reshape(
                8, P, 8 * SH))
        in_maps.append({"a_t": a_t, "xw1_in": xw1_in, "w2": W2})
    return in_maps


def _execute(in_maps, trace=False, tmpdir=None):
    # the first execution of a freshly loaded NEFF occasionally faults with
    # NRT_EXEC_UNIT_UNRECOVERABLE; a retry on a clean dispatch succeeds
    last = None
    for _ in range(3):
        try:
            return bass_utils.run_bass_kernel_spmd(
                _get_nc(), in_maps, core_ids=list(range(NCORES)),
                trace=trace, tmpdir=tmpdir)
        except Exception as e:  # noqa: BLE001
            last = e
    raise last


def _assemble(res):
    shards = [np.asarray(res.results[c]["out"]).astype(np.float32)
              for c in range(NCORES)]
    return np.concatenate(shards, axis=0)


def kernel(norm_adj_matrix, X, W1, W2):
    in_maps = _prep_in_maps(norm_adj_matrix, X, W1, W2)
    res = _execute(in_maps)
    return _assemble(res)

